# revision 23
# baseline (speedup 1.0000x reference)
"""SSIM loss kernel for Trainium2 (8 NeuronCores, batch-sharded data parallel).

Algorithm (per core, batch shard of 2 images = 6 channel-plane pairs):
  - 4 conv quantities per pair: x, y, x*y, x^2+y^2 (fp16)
  - separable 11-tap Gaussian conv as two PE matmul passes:
      pass1 (rows):  out1[col, j] = sum_k data[k, col] * A[k, j]   (data as lhsT
             -> output lands transposed: image cols in PSUM partitions)
      pass2 (cols):  out2[jj, r] = sum_k A[k, jj] * P1sb[k, r]     (A as lhsT
             -> back to row-major; final SSIM map orientation-invariant anyway)
    Row/col tiling: 5 halo tiles of 118 owned rows/cols (+5 halo each side).
  - fused elementwise epilogue (DVE/ACT/GPSIMD) + free-dim reduce into
    per-partition accumulators; host sums the 8x[128x8] partials.

Numerics: fp16 taps are ULP-tweaked so sum(taps)==1 to ~1e-9 (the SSIM
covariance cancellations amplify kernel DC-gain error ~25x; raw fp16
rounding of the taps alone costs 2e-2 relative error on the final mean).
"""

import numpy as np

WS = 11
SIGMA = 1.5
C1 = np.float64(0.01**2)
C2 = np.float64(0.03**2)

B, CH, H, W = 16, 3, 512, 512
N_CORES = 8
PLANES_PER_CORE = (B // N_CORES) * CH  # 6

# Halo tiling: 5 tiles of 118/118/118/118/40 owned rows, +-5 halo
OWN = [118, 118, 118, 118, 40]
OSTART = [0, 118, 236, 354, 472]
HSTART = [0, 113, 231, 349, 467]   # first input row/col of each halo tile
KP = [123, 128, 128, 128, 45]      # valid partitions (input rows/cols) per tile

_CACHE = {}


def _gauss_taps_fp16():
    x = np.arange(WS, dtype=np.float64)
    g = np.exp(-((x - WS / 2) ** 2) / (2.0 * SIGMA**2))
    g = (g / g.sum()).astype(np.float32)
    gq = g.astype(np.float16).astype(np.float64)
    # distribute ULP-level tweaks so the fp64 sum of fp16 taps is ~exactly 1
    for _ in range(200):
        resid = gq.sum() - 1.0
        if abs(resid) < 1e-9:
            break
        best, bi = None, None
        for i in range(WS):
            v = np.float16(gq[i])
            nxt = np.float64(np.nextafter(v, np.float16(-np.sign(resid) * 6e4)))
            step = nxt - gq[i]
            if abs(step) <= abs(resid) * 1.001 and (best is None or abs(step) > abs(best)):
                best, bi = step, i
        if bi is None:
            break
        gq[bi] += best
    return gq.astype(np.float16)


def _weight_mats():
    """A_mid[k, jj] = g[k-jj]   (band 0<=k-jj<=10), shape [128, 118]
       A_first[k, jj] = g[k-jj+5] (band -5<=k-jj<=5), shape [128, 118]"""
    g = _gauss_taps_fp16()
    a_mid = np.zeros((128, 118), np.float16)
    a_first = np.zeros((128, 118), np.float16)
    for k in range(128):
        for jj in range(118):
            d = k - jj
            if 0 <= d <= 10:
                a_mid[k, jj] = g[d]
            if -5 <= d <= 5:
                a_first[k, jj] = g[d + 5]
    return a_first, a_mid


def _build_program():
    from concourse import bass, bacc, tile
    import concourse.mybir as mybir

    dt = mybir.dt
    Alu = mybir.AluOpType
    Act = mybir.ActivationFunctionType

    nc = bacc.Bacc("TRN2", target_bir_lowering=False, debug=False)

    x_d = nc.dram_tensor("x", [PLANES_PER_CORE, H, W], dt.float16, kind="ExternalInput").ap()
    y_d = nc.dram_tensor("y", [PLANES_PER_CORE, H, W], dt.float16, kind="ExternalInput").ap()
    af_d = nc.dram_tensor("a_first", [128, 118], dt.float16, kind="ExternalInput").ap()
    am_d = nc.dram_tensor("a_mid", [128, 118], dt.float16, kind="ExternalInput").ap()
    on_d = nc.dram_tensor("ones", [128, 1], dt.float16, kind="ExternalInput").ap()
    out_d = nc.dram_tensor("out", [1, 512], dt.float32, kind="ExternalOutput").ap()

    f32, f16 = dt.float32, dt.float16
    c1 = float(C1)
    c2 = float(C2)

    with tile.TileContext(nc) as tc:
        with (
            tc.tile_pool(name="wts", bufs=1) as wpool,
            tc.tile_pool(name="qt", bufs=2) as qpool,
            tc.tile_pool(name="sq", bufs=2) as sqpool,
            tc.tile_pool(name="p1sb", bufs=2) as p1pool,
            tc.tile_pool(name="epi", bufs=2) as epool,
            tc.tile_pool(name="accp", bufs=1) as apool,
            tc.tile_pool(name="p1ps", bufs=2, space="PSUM") as psum1,
            tc.tile_pool(name="p2ps", bufs=1, space="PSUM") as psum2,
            tc.tile_pool(name="redps", bufs=1, space="PSUM") as psumr,
        ):
            a_first = wpool.tile([128, 118], f16, tag="af")
            a_mid = wpool.tile([128, 118], f16, tag="am")
            ones = wpool.tile([128, 1], f16, tag="ones")
            nc.sync.dma_start(out=a_first[:, :], in_=af_d[:, :])
            nc.sync.dma_start(out=a_mid[:, :], in_=am_d[:, :])
            nc.sync.dma_start(out=ones[:, :], in_=on_d[:, :])
            red = psumr.tile([128, 512], f32, tag="red")
            n_red = [0]

            def pass1_A(t):
                # weight (as rhs) for row-tile t of pass 1 / chunk t of pass 2
                if t == 0:
                    return a_first[0:123, 0:118]
                if t == 4:
                    return a_mid[0:45, 0:40]
                return a_mid[0:128, 0:118]

            for pair in range(PLANES_PER_CORE):
                # ---- load (fp16 from host) + products, batched FD=2560 ---
                xb = qpool.tile([128, 2560], f16, tag="xb")
                yb = qpool.tile([128, 2560], f16, tag="yb")
                ub = qpool.tile([128, 2560], f16, tag="ub")
                wb = qpool.tile([128, 2560], f16, tag="wb")
                sx = sqpool.tile([128, 2560], f16, tag="sx")
                sy = sqpool.tile([128, 2560], f16, tag="sy")
                # fill all 128 partitions with real rows (t0: rows 0..127;
                # t4: rows 467..511 + filler rows) so the batched product
                # ops never read uninitialized memory
                for t in range(5):
                    k = 128 if t < 4 else KP[4]
                    nc.sync.dma_start(out=xb[0:k, t * 512:(t + 1) * 512],
                                      in_=x_d[pair, HSTART[t]:HSTART[t] + k, :])
                    nc.sync.dma_start(out=yb[0:k, t * 512:(t + 1) * 512],
                                      in_=y_d[pair, HSTART[t]:HSTART[t] + k, :])
                nc.sync.dma_start(out=xb[45:128, 4 * 512:5 * 512], in_=x_d[pair, 0:83, :])
                nc.sync.dma_start(out=yb[45:128, 4 * 512:5 * 512], in_=y_d[pair, 0:83, :])
                nc.vector.tensor_tensor(out=ub[:, :], in0=xb[:, :], in1=yb[:, :], op=Alu.mult)
                nc.scalar.activation(out=sx[:, :], in_=xb[:, :], func=Act.Square)
                nc.scalar.activation(out=sy[:, :], in_=yb[:, :], func=Act.Square)
                nc.vector.tensor_tensor(out=wb[:, :], in0=sx[:, :], in1=sy[:, :], op=Alu.add)

                def qslice(qn, t, c0, cn):
                    base = {"M1": xb, "M2": yb, "S12": ub, "W": wb}[qn]
                    return base[0:KP[t], t * 512 + c0: t * 512 + c0 + cn]

                # ---- pass 1 (rows) + copy to SBUF ------------------------
                p1sb = {}
                copy_i = 0
                for qi, qn in enumerate(["M1", "M2", "S12", "W"]):
                    for c in range(5):
                        cw = KP[c]  # output partitions of this col chunk
                        p1 = psum1.tile([128, 512], f32, tag="p1")
                        for t in range(5):
                            k = KP[t]
                            nc.tensor.matmul(
                                out=p1[0:cw, OSTART[t]:OSTART[t] + OWN[t]],
                                lhsT=qslice(qn, t, HSTART[c], cw),
                                rhs=pass1_A(t),
                                start=True, stop=True,
                            )
                        sb = p1pool.tile([128, 512], f16, tag=f"p1sb_{qn}_{c}")
                        # copies rebalanced: ACT is cheaper per measured op and
                        # has less other work; give it 3 of every 4
                        if copy_i % 4 == 0:
                            nc.vector.tensor_copy(out=sb[0:cw, :], in_=p1[0:cw, :])
                        else:
                            nc.scalar.copy(out=sb[0:cw, :], in_=p1[0:cw, :])
                        copy_i += 1
                        p1sb[(qn, c)] = sb

                # ---- pass 2 (cols) + epilogue per chunk ------------------
                for c in range(5):
                    cw = KP[c]
                    p = OWN[c]  # partitions of final tiles
                    conv = {}
                    for qn in ["M1", "M2", "S12", "W"]:
                        o2 = psum2.tile([128, 512], f32, tag=f"o2_{qn}", name=f"o2_{qn}",
                                        bufs=2 if qn == "W" else 1)
                        nc.tensor.matmul(
                            out=o2[0:p, :],
                            lhsT=pass1_A(c),
                            rhs=p1sb[(qn, c)][0:cw, :],
                            start=True, stop=True,
                        )
                        conv[qn] = o2
                    M1, M2, S12, Wq = conv["M1"], conv["M2"], conv["S12"], conv["W"]

                    m2c = epool.tile([128, 512], f16, tag="m2c")
                    bt = epool.tile([128, 512], f16, tag="b")
                    ct = epool.tile([128, 512], f16, tag="c")
                    at = epool.tile([128, 512], f16, tag="a")
                    p1s = epool.tile([128, 512], f16, tag="p1s")
                    pm = epool.tile([128, 512], f16, tag="pm")
                    nr = epool.tile([128, 512], f16, tag="nr")
                    den = epool.tile([128, 512], f32, tag="den")
                    rden = epool.tile([128, 512], f32, tag="rden")
                    numt = epool.tile([128, 512], f32, tag="num")
                    ratio = epool.tile([128, 512], f16, tag="ratio")

                    # ACT: copy mu2 to SBUF, squares of the mu's
                    nc.scalar.copy(out=m2c[0:p, :], in_=M2[0:p, :])
                    nc.scalar.activation(out=bt[0:p, :], in_=M1[0:p, :], func=Act.Square)
                    nc.scalar.activation(out=ct[0:p, :], in_=m2c[0:p, :], func=Act.Square)
                    # DVE: a = mu1*mu2 (only one PSUM operand allowed)
                    nc.vector.tensor_tensor(out=at[0:p, :], in0=M1[0:p, :], in1=m2c[0:p, :], op=Alu.mult)
                    # p1s = (b + C1) + c = mu1^2 + mu2^2 + C1
                    nc.vector.scalar_tensor_tensor(
                        out=p1s[0:p, :], in0=bt[0:p, :], scalar=c1, in1=ct[0:p, :],
                        op0=Alu.add, op1=Alu.add)
                    # pm = (W + (C1+C2)) - p1s = sigma1+sigma2+C2
                    nc.vector.scalar_tensor_tensor(
                        out=pm[0:p, :], in0=Wq[0:p, :], scalar=c1 + c2, in1=p1s[0:p, :],
                        op0=Alu.add, op1=Alu.subtract)
                    # nr = (S12 + C2/2) - a = sigma12 + C2/2
                    nc.vector.scalar_tensor_tensor(
                        out=nr[0:p, :], in0=S12[0:p, :], scalar=c2 / 2, in1=at[0:p, :],
                        op0=Alu.add, op1=Alu.subtract)
                    # den = p1s * pm (fp32), rden = 1/den
                    nc.gpsimd.tensor_tensor(out=den[0:p, :], in0=p1s[0:p, :], in1=pm[0:p, :], op=Alu.mult)
                    nc.vector.reciprocal_approx_fast(out=rden[0:p, :], in_=den[0:p, :])
                    # num = (a + C1/2) * nr (fp32)
                    nc.vector.scalar_tensor_tensor(
                        out=numt[0:p, :], in0=at[0:p, :], scalar=c1 / 2, in1=nr[0:p, :],
                        op0=Alu.add, op1=Alu.mult)
                    # ratio = num * rden (fp16) ; reduce over partitions on PE
                    nc.gpsimd.tensor_tensor(out=ratio[0:p, :], in0=numt[0:p, :],
                                            in1=rden[0:p, :], op=Alu.mult)
                    nc.tensor.matmul(
                        out=red[0:1, :], lhsT=ones[0:p, :], rhs=ratio[0:p, :],
                        start=(n_red[0] == 0), stop=(n_red[0] == 29),
                        skip_group_check=True)
                    n_red[0] += 1

            # ---- write the [1, 512] column-sum accumulator ---------------
            redsb = apool.tile([128, 512], f32, tag="redsb")
            nc.vector.tensor_copy(out=redsb[0:1, :], in_=red[0:1, :])
            nc.sync.dma_start(out=out_d[0:1, :], in_=redsb[0:1, :])

    nc.compile()
    return nc


def _get_program():
    if "nc" not in _CACHE:
        _CACHE["nc"] = _build_program()
        _CACHE["weights"] = _weight_mats()
    return _CACHE["nc"], _CACHE["weights"]


def _in_maps(img1, img2):
    img1 = np.ascontiguousarray(np.asarray(img1, np.float32))
    img2 = np.ascontiguousarray(np.asarray(img2, np.float32))
    a_first, a_mid = _CACHE["weights"]
    per = B // N_CORES
    maps = []
    for core in range(N_CORES):
        s = slice(core * per, (core + 1) * per)
        maps.append({
            "x": img1[s].reshape(PLANES_PER_CORE, H, W).astype(np.float16),
            "y": img2[s].reshape(PLANES_PER_CORE, H, W).astype(np.float16),
            "a_first": a_first,
            "a_mid": a_mid,
            "ones": np.ones((128, 1), np.float16),
        })
    return maps


def kernel(img1, img2):
    from concourse import bass_utils

    nc, _ = _get_program()
    maps = _in_maps(img1, img2)
    res = bass_utils.run_bass_kernel_spmd(nc, maps, core_ids=list(range(N_CORES)))
    total = np.float64(0.0)
    for r in res.results:
        total += np.asarray(r["out"], np.float64).sum()
    return np.float32(4.0 * total / (B * CH * H * W))


# revision 34
# speedup vs baseline: 1.1828x; 1.1828x over previous
"""SSIM loss kernel for Trainium2 (8 NeuronCores, batch-sharded data parallel).

Algorithm (per core, batch shard of 2 images = 6 channel-plane pairs):
  - 4 conv quantities per pair: x, y, x*y, x^2+y^2 (fp16)
  - separable 11-tap Gaussian conv as two PE matmul passes:
      pass1 (rows):  out1[col, j] = sum_k data[k, col] * A[k, j]   (data as lhsT
             -> output lands transposed: image cols in PSUM partitions)
      pass2 (cols):  out2[jj, r] = sum_k A[k, jj] * P1sb[k, r]     (A as lhsT
             -> back to row-major; final SSIM map orientation-invariant anyway)
    Row/col tiling: 5 halo tiles of 118 owned rows/cols (+5 halo each side).
  - fused elementwise epilogue (DVE/ACT/GPSIMD) + free-dim reduce into
    per-partition accumulators; host sums the 8x[128x8] partials.

Numerics: fp16 taps are ULP-tweaked so sum(taps)==1 to ~1e-9 (the SSIM
covariance cancellations amplify kernel DC-gain error ~25x; raw fp16
rounding of the taps alone costs 2e-2 relative error on the final mean).
"""

import numpy as np

WS = 11
SIGMA = 1.5
C1 = np.float64(0.01**2)
C2 = np.float64(0.03**2)

B, CH, H, W = 16, 3, 512, 512
N_CORES = 8
PLANES_PER_CORE = (B // N_CORES) * CH  # 6

# Halo tiling: 5 tiles of 118/118/118/118/40 owned rows; every tile loads a
# full 128 input rows/cols (t4 shifted back to rows 384..511) so all matmuls
# are uniform [K=128, M<=128] and FWL-eligible; band alignment differs per
# tile via three weight matrices (A_first / A_mid / A_late).
OWN = [118, 118, 118, 118, 40]
OSTART = [0, 118, 236, 354, 472]
HSTART = [0, 113, 231, 349, 384]   # first input row/col of each (128-row) tile

_CACHE = {}


def _gauss_taps_fp16():
    x = np.arange(WS, dtype=np.float64)
    g = np.exp(-((x - WS / 2) ** 2) / (2.0 * SIGMA**2))
    g = (g / g.sum()).astype(np.float32)
    gq = g.astype(np.float16).astype(np.float64)
    # distribute ULP-level tweaks so the fp64 sum of fp16 taps is ~exactly 1
    for _ in range(200):
        resid = gq.sum() - 1.0
        if abs(resid) < 1e-9:
            break
        best, bi = None, None
        for i in range(WS):
            v = np.float16(gq[i])
            nxt = np.float64(np.nextafter(v, np.float16(-np.sign(resid) * 6e4)))
            step = nxt - gq[i]
            if abs(step) <= abs(resid) * 1.001 and (best is None or abs(step) > abs(best)):
                best, bi = step, i
        if bi is None:
            break
        gq[bi] += best
    return gq.astype(np.float16)


def _weight_mats():
    """A_first[k, jj] = g[k-jj+5]  (tile start 0,    out start 0)
       A_mid[k, jj]   = g[k-jj]    (tile start own-5, centered band +5)
       A_late[k, jj]  = g[k-jj-83] (tile rows 384.., out cols 472..)"""
    g = _gauss_taps_fp16()
    a_first = np.zeros((128, 118), np.float16)
    a_mid = np.zeros((128, 118), np.float16)
    a_late = np.zeros((128, 40), np.float16)
    for k in range(128):
        for jj in range(118):
            d = k - jj
            if -5 <= d <= 5:
                a_first[k, jj] = g[d + 5]
            if 0 <= d <= 10:
                a_mid[k, jj] = g[d]
        for jj in range(40):
            d = k - jj - 83
            if 0 <= d <= 10:
                a_late[k, jj] = g[d]
    return a_first, a_mid, a_late


def _build_program():
    from concourse import bass, bacc, tile
    import concourse.mybir as mybir

    dt = mybir.dt
    Alu = mybir.AluOpType
    Act = mybir.ActivationFunctionType

    nc = bacc.Bacc("TRN2", target_bir_lowering=False, debug=False)

    x_d = nc.dram_tensor("x", [PLANES_PER_CORE, H, W], dt.float16, kind="ExternalInput").ap()
    y_d = nc.dram_tensor("y", [PLANES_PER_CORE, H, W], dt.float16, kind="ExternalInput").ap()
    af_d = nc.dram_tensor("a_first", [128, 118], dt.float16, kind="ExternalInput").ap()
    am_d = nc.dram_tensor("a_mid", [128, 118], dt.float16, kind="ExternalInput").ap()
    al_d = nc.dram_tensor("a_late", [128, 40], dt.float16, kind="ExternalInput").ap()
    on_d = nc.dram_tensor("ones", [128, 1], dt.float16, kind="ExternalInput").ap()
    out_d = nc.dram_tensor("out", [1, 512], dt.float32, kind="ExternalOutput").ap()

    f32, f16 = dt.float32, dt.float16
    c1 = float(C1)
    c2 = float(C2)

    with tile.TileContext(nc) as tc:
        with (
            tc.tile_pool(name="wts", bufs=1) as wpool,
            tc.tile_pool(name="qt", bufs=2) as qpool,
            tc.tile_pool(name="sq", bufs=2) as sqpool,
            tc.tile_pool(name="p1sb", bufs=2) as p1pool,
            tc.tile_pool(name="epi", bufs=3) as epool,
            tc.tile_pool(name="accp", bufs=1) as apool,
            tc.tile_pool(name="p1ps", bufs=3, space="PSUM") as psum1,
            tc.tile_pool(name="p2ps", bufs=1, space="PSUM") as psum2,
            tc.tile_pool(name="redps", bufs=1, space="PSUM") as psumr,
        ):
            a_first = wpool.tile([128, 118], f16, tag="af")
            a_mid = wpool.tile([128, 118], f16, tag="am")
            a_late = wpool.tile([128, 40], f16, tag="al")
            ones = wpool.tile([128, 1], f16, tag="ones")
            nc.sync.dma_start(out=a_first[:, :], in_=af_d[:, :])
            nc.sync.dma_start(out=a_mid[:, :], in_=am_d[:, :])
            nc.sync.dma_start(out=a_late[:, :], in_=al_d[:, :])
            nc.sync.dma_start(out=ones[:, :], in_=on_d[:, :])
            red = psumr.tile([128, 512], f32, tag="red")
            n_red = [0]

            def pass1_A(t):
                # weight (as rhs) for row-tile t of pass 1 / chunk t of pass 2
                if t == 0:
                    return a_first[0:128, 0:118]
                if t == 4:
                    return a_late[0:128, 0:40]
                return a_mid[0:128, 0:118]

            for pair in range(PLANES_PER_CORE):
                # ---- load (fp16 from host) + products, batched FD=2560 ---
                xb = qpool.tile([128, 2560], f16, tag="xb")
                yb = qpool.tile([128, 2560], f16, tag="yb")
                ub = qpool.tile([128, 2560], f16, tag="ub")
                wb = qpool.tile([128, 2560], f16, tag="wb")
                sx = sqpool.tile([128, 2560], f16, tag="sx")
                sy = sqpool.tile([128, 2560], f16, tag="sy")
                for t in range(5):
                    nc.sync.dma_start(out=xb[0:128, t * 512:(t + 1) * 512],
                                      in_=x_d[pair, HSTART[t]:HSTART[t] + 128, :])
                    nc.sync.dma_start(out=yb[0:128, t * 512:(t + 1) * 512],
                                      in_=y_d[pair, HSTART[t]:HSTART[t] + 128, :])
                nc.vector.tensor_tensor(out=ub[:, :], in0=xb[:, :], in1=yb[:, :], op=Alu.mult)
                nc.scalar.activation(out=sx[:, :], in_=xb[:, :], func=Act.Square)
                nc.scalar.activation(out=sy[:, :], in_=yb[:, :], func=Act.Square)
                nc.vector.tensor_tensor(out=wb[:, :], in0=sx[:, :], in1=sy[:, :], op=Alu.add)

                def qslice(qn, t, c0):
                    base = {"M1": xb, "M2": yb, "S12": ub, "W": wb}[qn]
                    return base[0:128, t * 512 + c0: t * 512 + c0 + 128]

                # ---- pass 1 (rows) + copy to SBUF ------------------------
                p1sb = {}
                copy_i = 0
                for qi, qn in enumerate(["M1", "M2", "S12", "W"]):
                    for c in range(5):
                        p1 = psum1.tile([128, 512], f32, tag="p1")
                        for t in range(5):
                            nc.tensor.matmul(
                                out=p1[0:128, OSTART[t]:OSTART[t] + OWN[t]],
                                lhsT=qslice(qn, t, HSTART[c]),
                                rhs=pass1_A(t),
                                start=True, stop=True,
                            )
                        sb = p1pool.tile([128, 512], f16, tag=f"p1sb_{qn}_{c}")
                        # copies rebalanced: give ACT 3 of every 4
                        if copy_i % 4 == 0:
                            nc.vector.tensor_copy(out=sb[:, :], in_=p1[:, :])
                        else:
                            nc.scalar.copy(out=sb[:, :], in_=p1[:, :])
                        copy_i += 1
                        p1sb[(qn, c)] = sb

                # ---- pass 2 (cols) + epilogue per chunk ------------------
                for c in range(5):
                    p = OWN[c]  # partitions of final tiles
                    conv = {}
                    for qn in ["M1", "M2", "S12", "W"]:
                        o2 = psum2.tile([128, 512], f32, tag=f"o2_{qn}", name=f"o2_{qn}",
                                        bufs=1)
                        nc.tensor.matmul(
                            out=o2[0:p, :],
                            lhsT=pass1_A(c),
                            rhs=p1sb[(qn, c)][0:128, :],
                            start=True, stop=True,
                        )
                        conv[qn] = o2
                    M1, M2, S12, Wq = conv["M1"], conv["M2"], conv["S12"], conv["W"]

                    m2c = epool.tile([128, 512], f16, tag="m2c")
                    bt = epool.tile([128, 512], f16, tag="b")
                    ct = epool.tile([128, 512], f16, tag="c")
                    at = epool.tile([128, 512], f16, tag="a")
                    p1s = epool.tile([128, 512], f16, tag="p1s")
                    pm = epool.tile([128, 512], f16, tag="pm")
                    nr = epool.tile([128, 512], f16, tag="nr")
                    den = epool.tile([128, 512], f32, tag="den")
                    rden = epool.tile([128, 512], f32, tag="rden")
                    numt = epool.tile([128, 512], f32, tag="num")
                    ratio = epool.tile([128, 512], f16, tag="ratio")

                    # ACT: copy mu2 to SBUF, squares of the mu's
                    nc.scalar.copy(out=m2c[0:p, :], in_=M2[0:p, :])
                    nc.scalar.activation(out=bt[0:p, :], in_=M1[0:p, :], func=Act.Square)
                    nc.scalar.activation(out=ct[0:p, :], in_=m2c[0:p, :], func=Act.Square)
                    # DVE: a = mu1*mu2 (only one PSUM operand allowed)
                    nc.vector.tensor_tensor(out=at[0:p, :], in0=M1[0:p, :], in1=m2c[0:p, :], op=Alu.mult)
                    # p1s = (b + C1) + c = mu1^2 + mu2^2 + C1
                    nc.vector.scalar_tensor_tensor(
                        out=p1s[0:p, :], in0=bt[0:p, :], scalar=c1, in1=ct[0:p, :],
                        op0=Alu.add, op1=Alu.add)
                    # pm = (W + (C1+C2)) - p1s = sigma1+sigma2+C2
                    nc.vector.scalar_tensor_tensor(
                        out=pm[0:p, :], in0=Wq[0:p, :], scalar=c1 + c2, in1=p1s[0:p, :],
                        op0=Alu.add, op1=Alu.subtract)
                    # nr = (S12 + C2/2) - a = sigma12 + C2/2
                    nc.vector.scalar_tensor_tensor(
                        out=nr[0:p, :], in0=S12[0:p, :], scalar=c2 / 2, in1=at[0:p, :],
                        op0=Alu.add, op1=Alu.subtract)
                    # den = p1s * pm (fp32), rden = 1/den
                    nc.gpsimd.tensor_tensor(out=den[0:p, :], in0=p1s[0:p, :], in1=pm[0:p, :], op=Alu.mult)
                    nc.vector.reciprocal_approx_fast(out=rden[0:p, :], in_=den[0:p, :])
                    # num = (a + C1/2) * nr (fp32)
                    nc.vector.scalar_tensor_tensor(
                        out=numt[0:p, :], in0=at[0:p, :], scalar=c1 / 2, in1=nr[0:p, :],
                        op0=Alu.add, op1=Alu.mult)
                    # ratio = num * rden (fp16) ; reduce over partitions on PE
                    nc.gpsimd.tensor_tensor(out=ratio[0:p, :], in0=numt[0:p, :],
                                            in1=rden[0:p, :], op=Alu.mult)
                    nc.tensor.matmul(
                        out=red[0:1, :], lhsT=ones[0:p, :], rhs=ratio[0:p, :],
                        start=(n_red[0] == 0), stop=(n_red[0] == 29),
                        skip_group_check=True)
                    n_red[0] += 1

            # ---- write the [1, 512] column-sum accumulator ---------------
            redsb = apool.tile([128, 512], f32, tag="redsb")
            nc.vector.tensor_copy(out=redsb[0:1, :], in_=red[0:1, :])
            nc.sync.dma_start(out=out_d[0:1, :], in_=redsb[0:1, :])

    nc.compile()
    return nc


def _get_program():
    if "nc" not in _CACHE:
        _CACHE["nc"] = _build_program()
        _CACHE["weights"] = _weight_mats()
    return _CACHE["nc"], _CACHE["weights"]


def _in_maps(img1, img2):
    img1 = np.ascontiguousarray(np.asarray(img1, np.float32))
    img2 = np.ascontiguousarray(np.asarray(img2, np.float32))
    a_first, a_mid, a_late = _CACHE["weights"]
    per = B // N_CORES
    maps = []
    for core in range(N_CORES):
        s = slice(core * per, (core + 1) * per)
        maps.append({
            "x": img1[s].reshape(PLANES_PER_CORE, H, W).astype(np.float16),
            "y": img2[s].reshape(PLANES_PER_CORE, H, W).astype(np.float16),
            "a_first": a_first,
            "a_mid": a_mid,
            "a_late": a_late,
            "ones": np.ones((128, 1), np.float16),
        })
    return maps


def kernel(img1, img2):
    from concourse import bass_utils

    nc, _ = _get_program()
    maps = _in_maps(img1, img2)
    res = bass_utils.run_bass_kernel_spmd(nc, maps, core_ids=list(range(N_CORES)))
    total = np.float64(0.0)
    for r in res.results:
        total += np.asarray(r["out"], np.float64).sum()
    return np.float32(4.0 * total / (B * CH * H * W))
